# revision 1
# baseline (speedup 1.0000x reference)
"""Trainium2 Bass kernel for the gnn_message_passing problem.

Contract: kernel(**inputs) takes the FULL unsharded inputs (numpy, keyed as in
setup_inputs()) and returns the FULL output [16, 32, 100, 1024] float32.

Strategy: pure data parallel over batch*time (BT = 512 graphs) across 8
NeuronCores (64 graphs each). All math runs on device; the host only does
layout packing (transpose/cast/shard) and unpacking.

Per graph g (lf [100, 1024], gf [49, 1024]):
  rl[n] = 1/||lf[n]||, rg[m] = 1/||gf[m]||      (squares + ones-matmul)
  A_raw = (lf @ gf^T) * outer(rl, rg)           (norms folded after matmul)
  E     = exp(5 * A_raw), s = rowsum(E)
  y'    = [E | s*A_raw | s] @ [gf@W1^T ; W2^T ; b]   (stacked matmul)
  out   = rstd * LeakyReLU(y' - mean(y'))       (LN is invariant to the
            per-row scale s; LeakyReLU commutes with the positive rstd)

The fast path batches small ops over 8-graph blocks to amortize per-
instruction overheads and keeps the scalar engine to three activation-table
groups (Exp/Sqrt/Lrelu) to minimize activation-table reloads.

When W_adj is not exactly identity or ln_g/ln_b are not the identity affine,
a general (slower, fully honest) fallback kernel is built instead.
"""

import numpy as np
import ml_dtypes

B, T, N, C = 16, 32, 100, 1024
M = 49
MP = 64  # m padded to a 32-aligned slab
BT = B * T
NCORES = 8
GPC = BT // NCORES  # graphs per core (64)
QPC = GPC // 2  # graph pairs per core (32)
CT = C // 128  # contraction tiles (8)
G = 8  # graphs per block (fast path)
NBLK = GPC // G

_BF16 = ml_dtypes.bfloat16

# per-parity geometry of the stacked matmul
#  j=0: rhs rows = [gfW1(0:49) | W2T(49:98) | b(98)]            k = 99
#  j=1: rhs rows = [W2T(0:49) | b(49) | 0(50:64) | gfW1(64:113)]  k = 113
KJ = [2 * M + 1, MP + M]
E_COL = [0, MP]
ARAW_COL = [M, 0]
S_COL = [2 * M, M]


def _build_fast():
    import concourse.bacc as bacc
    import concourse.mybir as mybir
    import concourse.tile as tile
    from concourse import masks

    AF = mybir.ActivationFunctionType
    ALU = mybir.AluOpType
    bf16 = mybir.dt.bfloat16
    f32 = mybir.dt.float32

    nc = bacc.Bacc("TRN2", target_bir_lowering=False, debug=False,
                   num_devices=NCORES)

    lft = nc.dram_tensor("lft", [GPC, 128, CT, N], bf16, kind="ExternalInput")
    gfp = nc.dram_tensor("gfp", [QPC, 128, CT, 2, MP], bf16,
                         kind="ExternalInput")
    w1t = nc.dram_tensor("w1t", [128, CT, C], bf16, kind="ExternalInput")
    w2tb = nc.dram_tensor("w2tb", [M + 1, C], bf16, kind="ExternalInput")
    out = nc.dram_tensor("out", [GPC, N, C], f32, kind="ExternalOutput")

    with tile.TileContext(nc) as tc:
        with (
            tc.tile_pool(name="statics", bufs=1) as statics,
            tc.tile_pool(name="blk2", bufs=2) as blk2,
            tc.tile_pool(name="blk1", bufs=2) as blk1,
            tc.tile_pool(name="ps_p", bufs=1, space="PSUM") as ps_p,
            tc.tile_pool(name="ps_s", bufs=1, space="PSUM") as ps_s,
            tc.tile_pool(name="ps_n", bufs=1, space="PSUM") as ps_n,
            tc.tile_pool(name="ps_t", bufs=1, space="PSUM") as ps_t,
            tc.tile_pool(name="ps_pw", bufs=1, space="PSUM") as ps_pw,
            tc.tile_pool(name="ps_y", bufs=1, space="PSUM") as ps_y,
        ):
            # ---- static tiles ----
            ident_bf = statics.tile([128, 128], bf16)
            masks.make_identity(nc, ident_bf[:])
            onecol = statics.tile([128, 1], bf16)
            nc.gpsimd.memset(onecol[:], 1.0)
            epsln = statics.tile([128, 1], f32)
            nc.gpsimd.memset(epsln[:], 1e-5)
            w1t_sb = statics.tile([128, CT, C], bf16)
            nc.sync.dma_start(w1t_sb[:], w1t.ap())
            # one R-stack pair per in-flight block pair (4 pairs per block)
            rstk0s, rstk1s = [], []
            for qi in range(G // 2):
                r0 = statics.tile([2 * M + 1, C], bf16, name=f"rstk0_{qi}",
                                  tag=f"rstk0_{qi}")
                nc.sync.dma_start(r0[M:2 * M + 1, :], w2tb.ap())
                r1 = statics.tile([MP + M, C], bf16, name=f"rstk1_{qi}",
                                  tag=f"rstk1_{qi}")
                nc.gpsimd.memset(r1[0:MP, :], 0.0)
                nc.sync.dma_start(r1[0:M + 1, :], w2tb.ap())
                rstk0s.append(r0)
                rstk1s.append(r1)

            for bk in range(NBLK):
                # ---- loads ----
                lftb = blk2.tile([128, G, CT, N], bf16, tag="lftb")
                lsrc = lft.ap()[bk * G:(bk + 1) * G]  # [G, 128, CT, N]
                nc.sync.dma_start(
                    lftb[:], lsrc.rearrange("g p t n -> p g t n"))
                gfpb = blk2.tile([128, G // 2, CT, 2, MP], bf16, tag="gfpb")
                gsrc = gfp.ap()[bk * (G // 2):(bk + 1) * (G // 2)]
                nc.sync.dma_start(
                    gfpb[:], gsrc.rearrange("q p t j m -> p q t j m"))

                # ---- squared inputs (one DVE op each) ----
                sqlb = blk1.tile([128, CT, G, N], bf16, tag="sqlb", bufs=1)
                nc.vector.tensor_tensor(
                    out=sqlb[:].rearrange("p t g n -> p g t n"),
                    in0=lftb[:], in1=lftb[:], op=ALU.mult)
                sqgb = blk1.tile([128, CT, G // 2, 2, MP], bf16,
                                 tag="sqgb", bufs=1)
                nc.vector.tensor_tensor(
                    out=sqgb[:].rearrange("p t q j m -> p q t j m"),
                    in0=gfpb[:], in1=gfpb[:], op=ALU.mult)

                # ---- norm sums via ones-matmul chains ----
                # bankN rows: 0 -> s_l graphs 0..3, 32 -> s_l graphs 4..7,
                #             64 -> s_g all 4 pairs (with zero padding)
                bankN = ps_n.tile([128, 512], f32, tag="bankN")
                for ct in range(CT):
                    st = (ct == 0)
                    sp = (ct == CT - 1)
                    nc.tensor.matmul(bankN[0:1, 0:4 * N], onecol[:],
                                     sqlb[:, ct, 0:4, :], start=st, stop=sp)
                    nc.tensor.matmul(bankN[32:33, 0:4 * N], onecol[:],
                                     sqlb[:, ct, 4:8, :], start=st, stop=sp)
                    nc.tensor.matmul(bankN[64:65, 0:512], onecol[:],
                                     sqgb[:, ct, :, :, :], start=st, stop=sp)

                # raw sum rows -> bf16, gathered onto partition 0 (matmul
                # operands must share partitions): s_l at cols gi*100,
                # s_g at 800 + gi*49. The 1/sqrt happens later, elementwise
                # on the outer-product result (wide op instead of row ops).
                nraw = blk1.tile([128, 512], bf16, tag="nraw")
                nrow_flat = blk1.tile([1, 1200], bf16, tag="nrow_flat")
                nc.vector.tensor_copy(nrow_flat[0:1, 0:4 * N],
                                      bankN[0:1, 0:4 * N])
                nc.vector.tensor_copy(nraw[32:33, 0:4 * N],
                                      bankN[32:33, 0:4 * N])
                nc.vector.tensor_copy(nraw[64:65, 0:512],
                                      bankN[64:65, 0:512])
                nc.sync.dma_start(nrow_flat[0:1, 4 * N:8 * N],
                                  nraw[32:33, 0:4 * N])
                rgv = nraw[64:65, :].rearrange(
                    "p (q j m) -> p q j m", q=G // 2, j=2)
                nc.sync.dma_start(
                    nrow_flat[0:1, 8 * N:8 * N + G * M], rgv[:, :, :, 0:M])

                yo_blk = blk2.tile([N, G, C], f32, tag="yo_blk")
                bankP = ps_p.tile([128, G, M], f32, tag="bankP")
                bankS = ps_s.tile([128, G, M], f32, tag="bankS")
                stackb = blk2.tile([N, G, 128], bf16, tag="stackb")
                lhs_yb = blk2.tile([128, G, N], bf16, tag="lhs_yb")
                statsb = blk1.tile([N, G, 2, 6], f32, tag="statsb")
                mvb = blk1.tile([N, G, 2], f32, tag="mvb")
                negmub = blk1.tile([N, G], f32, tag="negmub")

                for qi in range(G // 2):
                    # ---- per-pair: gfW1 for both graphs ----
                    pw = ps_pw.tile([128, C], f32, tag="pw")
                    for ct in range(CT):
                        for h in range(2):
                            nc.tensor.matmul(
                                pw[:, h * 512:(h + 1) * 512],
                                gfpb[:, qi, ct, :, :],
                                w1t_sb[:, ct, h * 512:(h + 1) * 512],
                                start=(ct == 0), stop=(ct == CT - 1))

                    for j in range(2):
                        gi = 2 * qi + j
                        # P_raw and S
                        for ct in range(CT):
                            nc.tensor.matmul(
                                bankP[0:N, gi, :], lftb[:, gi, ct, :],
                                gfpb[:, qi, ct, j, 0:M],
                                start=(ct == 0), stop=(ct == CT - 1))
                        nc.tensor.matmul(
                            bankS[0:N, gi, :],
                            nrow_flat[0:1, gi * N:(gi + 1) * N],
                            nrow_flat[0:1, 8 * N + gi * M:
                                      8 * N + (gi + 1) * M],
                            start=True, stop=True)

                    # rstk gfW1 rows for this pair (no partition shift)
                    nc.vector.tensor_copy(rstk0s[qi][0:M, :], pw[0:M, :])
                    nc.vector.tensor_copy(rstk1s[qi][MP:MP + M, :],
                                          pw[MP:MP + M, :])

                # ---- batched softmax pieces ----
                # rs = 1/sqrt(s_l*s_g) elementwise on the outer products
                rs_f = blk1.tile([N, G, M], f32, tag="rs_f")
                nc.vector.reciprocal(rs_f[:], bankS[0:N, :, :])
                rs_s = blk1.tile([N, G, M], f32, tag="rs_s")
                nc.scalar.activation(rs_s[:], rs_f[:], AF.Sqrt)
                araw = blk1.tile([N, G, M], f32, tag="araw")
                nc.vector.tensor_tensor(out=araw[:], in0=bankP[0:N, :, :],
                                        in1=rs_s[:], op=ALU.mult)
                # E into stack (two parity groups), bf16
                for par in range(2):
                    nc.scalar.activation(
                        stackb[:, par::2, E_COL[par]:E_COL[par] + M],
                        araw[:, par::2, :], AF.Exp, scale=5.0)
                # row sums s
                ssumb = blk1.tile([N, 2, G // 2], f32, tag="ssumb")
                for par in range(2):
                    nc.vector.tensor_reduce(
                        out=ssumb[:, par, :],
                        in_=stackb[:, par::2, E_COL[par]:E_COL[par] + M],
                        axis=mybir.AxisListType.X, op=ALU.add)
                # s*A_raw and s columns, zero-pad odd slabs
                nc.gpsimd.memset(stackb[:, 1::2, M + 1:MP], 0.0)
                for gi in range(G):
                    j = gi % 2
                    sc = ssumb[:, j, gi // 2:gi // 2 + 1]
                    nc.vector.tensor_scalar_mul(
                        out=stackb[:, gi, ARAW_COL[j]:ARAW_COL[j] + M],
                        in0=araw[:, gi, :], scalar1=sc)
                    nc.gpsimd.tensor_copy(
                        out=stackb[:, gi, S_COL[j]:S_COL[j] + 1], in_=sc)

                bankT = ps_t.tile([128, G, N], bf16, tag="bankT")
                # ---- per-graph tail ----
                for gi in range(G):
                    j = gi % 2
                    kj = KJ[j]
                    g = bk * G + gi
                    nc.tensor.transpose(bankT[0:kj, gi, :],
                                        stackb[:, gi, 0:kj],
                                        ident_bf[0:N, 0:N])
                    nc.vector.tensor_copy(lhs_yb[0:kj, gi, :],
                                          bankT[0:kj, gi, :])

                    rstk_g = rstk0s[gi // 2] if j == 0 else rstk1s[gi // 2]
                    yps = ps_y.tile([N, C], f32, tag="y", name=f"y{gi}")
                    for h in range(2):
                        nc.tensor.matmul(
                            yps[:, h * 512:(h + 1) * 512],
                            lhs_yb[0:kj, gi, :],
                            rstk_g[:, h * 512:(h + 1) * 512],
                            start=True, stop=True)

                    yv = yps[:].rearrange("p (a b) -> p a b", a=2)
                    nc.vector.bn_stats(out=statsb[:, gi, 0, :],
                                       in_=yv[:, 0, :])
                    nc.vector.bn_stats(out=statsb[:, gi, 1, :],
                                       in_=yv[:, 1, :])
                    nc.vector.bn_aggr(out=mvb[:, gi, :],
                                      in_=statsb[:, gi, :, :])
                    nc.vector.tensor_scalar(
                        out=negmub[:, gi:gi + 1], in0=mvb[:, gi, 0:1],
                        scalar1=-1.0, scalar2=None, op0=ALU.mult)
                    nc.scalar.activation(yo_blk[:, gi, :], yps[:], AF.Lrelu,
                                         bias=negmub[:, gi:gi + 1],
                                         alpha=0.01)

                # ---- block-level rstd and final scale ----
                rstd_f = blk1.tile([N, G], f32, tag="rstd_f")
                nc.scalar.activation(rstd_f[:], mvb[:, :, 1:2], AF.Sqrt,
                                     bias=epsln[0:N])
                rstdb = blk1.tile([N, G], f32, tag="rstdb")
                nc.vector.reciprocal(rstdb[:], rstd_f[:])
                for gi in range(G):
                    nc.vector.tensor_scalar_mul(
                        out=yo_blk[:, gi, :], in0=yo_blk[:, gi, :],
                        scalar1=rstdb[:, gi:gi + 1])
                dsts = out.ap()[bk * G:(bk + 1) * G]  # [G, N, C]
                nc.sync.dma_start(dsts.rearrange("g n c -> n g c"),
                                  yo_blk[:])

    nc.compile()
    return nc


def _build_fallback(general_w: bool, general_ln: bool):
    import concourse.bacc as bacc
    import concourse.mybir as mybir
    import concourse.tile as tile
    from concourse import masks

    AF = mybir.ActivationFunctionType
    ALU = mybir.AluOpType
    bf16 = mybir.dt.bfloat16
    f32 = mybir.dt.float32

    nc = bacc.Bacc("TRN2", target_bir_lowering=False, debug=False,
                   num_devices=NCORES)

    lft = nc.dram_tensor("lft", [GPC, 128, CT, N], bf16, kind="ExternalInput")
    gfp = nc.dram_tensor("gfp", [QPC, 128, CT, 2, MP], bf16,
                         kind="ExternalInput")
    w1t = nc.dram_tensor("w1t", [128, CT, C], bf16, kind="ExternalInput")
    w2tb = nc.dram_tensor("w2tb", [M + 1, C], bf16, kind="ExternalInput")
    if general_w:
        wadjt = nc.dram_tensor("wadjt", [128, CT, CT, 128], bf16,
                               kind="ExternalInput")
    if general_ln:
        grep = nc.dram_tensor("grep", [128, C], f32, kind="ExternalInput")
        brep = nc.dram_tensor("brep", [128, C], f32, kind="ExternalInput")
    out = nc.dram_tensor("out", [GPC, N, C], f32, kind="ExternalOutput")

    with tile.TileContext(nc) as tc:
        with (
            tc.tile_pool(name="statics", bufs=1) as statics,
            tc.tile_pool(name="pair_sb", bufs=2) as pair_sb,
            tc.tile_pool(name="graph_sb", bufs=3) as graph_sb,
            tc.tile_pool(name="ps_small", bufs=2, space="PSUM") as ps_small,
            tc.tile_pool(name="ps_pair", bufs=1, space="PSUM") as ps_pair,
            tc.tile_pool(name="ps_y", bufs=1 if general_w else 2,
                         space="PSUM") as ps_y,
        ):
            ident = statics.tile([128, 128], f32)
            masks.make_identity(nc, ident[:])
            onecol = statics.tile([128, 1], bf16)
            nc.gpsimd.memset(onecol[:], 1.0)
            epsln = statics.tile([128, 1], f32)
            nc.gpsimd.memset(epsln[:], 1e-5)
            w1t_sb = statics.tile([128, CT, C], bf16)
            nc.sync.dma_start(w1t_sb[:], w1t.ap())
            rstk0 = statics.tile([2 * M + 1, C], bf16)
            nc.sync.dma_start(rstk0[M:2 * M + 1, :], w2tb.ap())
            rstk1 = statics.tile([MP + M, C], bf16)
            nc.gpsimd.memset(rstk1[0:MP, :], 0.0)
            nc.sync.dma_start(rstk1[0:M + 1, :], w2tb.ap())
            rstk = [rstk0, rstk1]
            if general_w:
                wadj_sb = statics.tile([128, CT, CT, 128], bf16)
                nc.sync.dma_start(wadj_sb[:], wadjt.ap())
            if general_ln:
                grep_sb = statics.tile([128, C], f32)
                brep_sb = statics.tile([128, C], f32)
                nc.sync.dma_start(grep_sb[:], grep.ap())
                nc.sync.dma_start(brep_sb[:], brep.ap())

            for q in range(QPC):
                gfp_t = pair_sb.tile([128, CT, 2, MP], bf16, tag="gfp")
                nc.sync.dma_start(gfp_t[:], gfp.ap()[q])

                pw = ps_pair.tile([128, C], f32, tag="pw")
                for ct in range(CT):
                    for h in range(2):
                        nc.tensor.matmul(
                            pw[:, h * 512:(h + 1) * 512],
                            gfp_t[:, ct, :, :],
                            w1t_sb[:, ct, h * 512:(h + 1) * 512],
                            start=(ct == 0), stop=(ct == CT - 1))

                if general_w:
                    qps = ps_pair.tile([128, CT, 2, MP], f32, tag="qps")
                    for dt_i in range(CT):
                        for ct in range(CT):
                            nc.tensor.matmul(
                                qps[:, dt_i, :, :],
                                wadj_sb[:, ct, dt_i, :],
                                gfp_t[:, ct, :, :],
                                start=(ct == 0), stop=(ct == CT - 1))
                    qp_sb = pair_sb.tile([128, CT, 2, MP], bf16, tag="qp")
                    nc.scalar.activation(qp_sb[:], qps[:], AF.Copy)
                    rhs_pm = qp_sb
                else:
                    rhs_pm = gfp_t

                sqg = pair_sb.tile([128, CT, 2, MP], bf16, tag="sqg")
                nc.vector.tensor_tensor(
                    out=sqg[:], in0=gfp_t[:], in1=gfp_t[:], op=ALU.mult)
                rg_ps = ps_small.tile([128, 512], f32, tag="sm")
                for ct in range(CT):
                    nc.tensor.matmul(
                        rg_ps[0:1, 0:2 * MP], onecol[:], sqg[:, ct, :, :],
                        start=(ct == 0), stop=(ct == CT - 1))
                rg_f = pair_sb.tile([1, 2, MP], f32, tag="rgf")
                nc.vector.reciprocal(rg_f[:, 0, 0:M], rg_ps[0:1, 0:M])
                nc.vector.reciprocal(rg_f[:, 1, 0:M],
                                     rg_ps[0:1, MP:MP + M])
                rg_row = pair_sb.tile([1, 2, MP], bf16, tag="rgr")
                nc.scalar.activation(rg_row[:, 0, 0:M], rg_f[:, 0, 0:M],
                                     AF.Sqrt)
                nc.scalar.activation(rg_row[:, 1, 0:M], rg_f[:, 1, 0:M],
                                     AF.Sqrt)

                for j in range(2):
                    g = 2 * q + j
                    kj = KJ[j]
                    lft_t = graph_sb.tile([128, CT, N], bf16, tag="lft")
                    nc.sync.dma_start(lft_t[:], lft.ap()[g])

                    sql = graph_sb.tile([128, CT, N], bf16, tag="sql")
                    nc.vector.tensor_tensor(
                        out=sql[:], in0=lft_t[:], in1=lft_t[:], op=ALU.mult)
                    sm = ps_small.tile([128, 512], f32, tag="sm")
                    for ct in range(CT):
                        nc.tensor.matmul(
                            sm[0:1, 256:256 + N], onecol[:], sql[:, ct, :],
                            start=(ct == 0), stop=(ct == CT - 1))
                    sl_f = graph_sb.tile([1, N], f32, tag="slf")
                    nc.vector.reciprocal(sl_f[:], sm[0:1, 256:256 + N])
                    rl_row = graph_sb.tile([1, N], bf16, tag="rlr")
                    nc.scalar.activation(rl_row[:], sl_f[:], AF.Sqrt)

                    nc.tensor.matmul(
                        sm[0:N, 64:64 + M], rl_row[:],
                        rg_row[:, j, 0:M], start=True, stop=True)
                    s_sb = graph_sb.tile([N, M], f32, tag="s_sb")
                    nc.scalar.activation(s_sb[:], sm[0:N, 64:64 + M],
                                         AF.Copy)

                    for ct in range(CT):
                        nc.tensor.matmul(
                            sm[0:N, 0:M], lft_t[:, ct, :],
                            rhs_pm[:, ct, j, 0:M],
                            start=(ct == 0), stop=(ct == CT - 1))

                    stack = graph_sb.tile([N, 128], f32, tag="stack")
                    araw = stack[:, ARAW_COL[j]:ARAW_COL[j] + M]
                    nc.vector.tensor_tensor(
                        out=araw, in0=sm[0:N, 0:M], in1=s_sb[:],
                        op=ALU.mult)
                    nc.gpsimd.memset(stack[:, S_COL[j]:S_COL[j] + 1], 1.0)
                    if j == 1:
                        nc.gpsimd.memset(stack[:, M + 1:MP], 0.0)

                    e_t = graph_sb.tile([N, M], f32, tag="e")
                    ssum = graph_sb.tile([N, 1], f32, tag="ssum")
                    nc.scalar.activation(
                        e_t[:], araw, AF.Exp, scale=5.0, accum_out=ssum[:])
                    sinv = graph_sb.tile([N, 1], f32, tag="sinv")
                    nc.vector.reciprocal(sinv[:], ssum[:])
                    nc.vector.tensor_scalar(
                        out=stack[:, E_COL[j]:E_COL[j] + M], in0=e_t[:],
                        scalar1=sinv[:], scalar2=None, op0=ALU.mult)

                    ident_b = graph_sb.tile([128, 128], bf16, tag="idb")
                    nc.vector.tensor_copy(ident_b[0:N, 0:N], ident[0:N, 0:N])
                    stack_b = graph_sb.tile([N, 128], bf16, tag="stackb")
                    nc.vector.tensor_copy(stack_b[:, 0:kj], stack[:, 0:kj])
                    nc.tensor.transpose(
                        sm[0:kj, 128:128 + N], stack_b[:, 0:kj],
                        ident_b[0:N, 0:N])
                    lhs_y = graph_sb.tile([128, N], bf16, tag="lhy")
                    nc.scalar.activation(
                        lhs_y[0:kj, :], sm[0:kj, 128:128 + N], AF.Copy)

                    if j == 0:
                        nc.scalar.activation(
                            rstk0[0:M, :], pw[0:M, :], AF.Copy)
                    else:
                        nc.scalar.activation(
                            rstk1[MP:MP + M, :], pw[MP:MP + M, :], AF.Copy)

                    yps = ps_y.tile([N, C], f32, tag="y")
                    for h in range(2):
                        nc.tensor.matmul(
                            yps[:, h * 512:(h + 1) * 512], lhs_y[0:kj, :],
                            rstk[j][:, h * 512:(h + 1) * 512],
                            start=True, stop=True)

                    stats = graph_sb.tile([N, 2, 6], f32, tag="stats")
                    yps_v = yps[:].rearrange("p (a b) -> p a b", a=2)
                    nc.vector.bn_stats(out=stats[:, 0, :], in_=yps_v[:, 0, :])
                    nc.vector.bn_stats(out=stats[:, 1, :], in_=yps_v[:, 1, :])
                    mv = graph_sb.tile([N, 2], f32, tag="mv")
                    nc.vector.bn_aggr(out=mv[:], in_=stats[:])
                    rstd = graph_sb.tile([N, 1], f32, tag="rstd")
                    nc.scalar.activation(
                        rstd[:], mv[:, 1:2], AF.Sqrt, bias=epsln[0:N])
                    nc.vector.reciprocal(rstd[:], rstd[:])
                    negmurs = graph_sb.tile([N, 1], f32, tag="negmurs")
                    nc.vector.tensor_scalar(
                        out=negmurs[:], in0=mv[:, 0:1], scalar1=rstd[:],
                        scalar2=-1.0, op0=ALU.mult, op1=ALU.mult)

                    y_out = graph_sb.tile([N, C], f32, tag="yo")
                    if general_ln:
                        nc.scalar.activation(
                            y_out[:], yps[:], AF.Copy, bias=negmurs[:],
                            scale=rstd[:])
                        nc.vector.tensor_tensor(
                            out=y_out[:], in0=y_out[:], in1=grep_sb[0:N, :],
                            op=ALU.mult)
                        nc.vector.tensor_tensor(
                            out=y_out[:], in0=y_out[:], in1=brep_sb[0:N, :],
                            op=ALU.add)
                        nc.scalar.activation(
                            y_out[:], y_out[:], AF.Lrelu, alpha=0.01)
                    else:
                        nc.scalar.activation(
                            y_out[:], yps[:], AF.Lrelu, bias=negmurs[:],
                            scale=rstd[:], alpha=0.01)
                    nc.sync.dma_start(out.ap()[g], y_out[:])

    nc.compile()
    return nc


_cache = {}


def _get_nc(general_w: bool, general_ln: bool):
    key = (general_w, general_ln)
    if key not in _cache:
        if general_w or general_ln:
            _cache[key] = _build_fallback(general_w, general_ln)
        else:
            _cache[key] = _build_fast()
    return _cache[key]


def _pack_inputs(local_feat, global_feat, W_aff, b_aff):
    lf = np.ascontiguousarray(local_feat.reshape(BT, N, C))
    gf = np.ascontiguousarray(global_feat.reshape(BT, M, C))
    # lft[g, p, t, n] = lf[g, n, t*128+p]
    lft = lf.transpose(0, 2, 1).reshape(BT, CT, 128, N).transpose(0, 2, 1, 3)
    lft = np.ascontiguousarray(lft.astype(_BF16))
    # gfp[q, p, t, j, m] = gf[2q+j, m, t*128+p], m zero-padded 49 -> 64
    gfp = np.zeros((BT // 2, 128, CT, 2, MP), dtype=_BF16)
    g4 = gf.transpose(0, 2, 1).reshape(BT // 2, 2, CT, 128, M)
    gfp[:, :, :, :, 0:M] = g4.transpose(0, 3, 2, 1, 4).astype(_BF16)
    # w1t[p, t, co] = W_aff[co, t*128+p]
    w1t = np.ascontiguousarray(
        W_aff[:, :C].T.reshape(CT, 128, C).transpose(1, 0, 2).astype(_BF16))
    # w2tb rows 0:49 = W2^T, row 49 = b_aff
    w2tb = np.concatenate([W_aff[:, C:C + M].T, b_aff[None, :]], axis=0)
    w2tb = np.ascontiguousarray(w2tb.astype(_BF16))
    return lft, gfp, w1t, w2tb


def _make_in_maps(lft, gfp, w1t, w2tb, extra):
    shared = {"w1t": w1t, "w2tb": w2tb, **extra}
    in_maps = []
    for k in range(NCORES):
        gs = slice(k * GPC, (k + 1) * GPC)
        qs = slice(k * QPC, (k + 1) * QPC)
        in_maps.append({"lft": np.ascontiguousarray(lft[gs]),
                        "gfp": np.ascontiguousarray(gfp[qs]), **shared})
    return in_maps


def kernel(local_feat, global_feat, pos, W_adj, W_aff, b_aff, ln_g, ln_b):
    from concourse.bass_utils import run_bass_kernel_spmd

    general_w = not np.array_equal(W_adj, np.eye(C, dtype=W_adj.dtype))
    general_ln = not (np.all(ln_g == 1.0) and np.all(ln_b == 0.0))

    lft, gfp, w1t, w2tb = _pack_inputs(local_feat, global_feat, W_aff, b_aff)

    extra = {}
    if general_w:
        # wadjt[p, ct, dt, d] = W_adj[dt*128+d, ct*128+p]
        wadjt = W_adj.T.reshape(CT, 128, CT, 128).transpose(1, 0, 2, 3)
        extra["wadjt"] = np.ascontiguousarray(wadjt.astype(_BF16))
    if general_ln:
        extra["grep"] = np.ascontiguousarray(
            np.broadcast_to(ln_g[None, :], (128, C)).astype(np.float32))
        extra["brep"] = np.ascontiguousarray(
            np.broadcast_to(ln_b[None, :], (128, C)).astype(np.float32))

    nc = _get_nc(general_w, general_ln)
    in_maps = _make_in_maps(lft, gfp, w1t, w2tb, extra)

    res = run_bass_kernel_spmd(nc, in_maps, core_ids=list(range(NCORES)))
    y = np.concatenate([res.results[k]["out"] for k in range(NCORES)], axis=0)
    return np.ascontiguousarray(y.reshape(B, T, N, C).astype(np.float32))



# revision 38
# speedup vs baseline: 1.8980x; 1.8980x over previous
"""Trainium2 Bass kernel for the gnn_message_passing problem.

Contract: kernel(**inputs) takes the FULL unsharded inputs (numpy, keyed as in
setup_inputs()) and returns the FULL output [16, 32, 100, 1024] float32.

Strategy: pure data parallel over batch*time (BT = 512 graphs) across 8
NeuronCores (64 graphs each). All math runs on device; the host only does
layout packing (transpose/cast/shard) and unpacking.

Per graph g (lf [100, 1024], gf [49, 1024]):
  rl[n] = 1/||lf[n]||, rg[m] = 1/||gf[m]||
  A_raw = (lf @ gf^T) * outer(rl, rg)
  E     = exp(5 * A_raw), s = rowsum(E)
  y'    = E @ (gf@W1^T) + (s*A_raw) @ W2^T      (b == 0 specialization;
            y' = s * y up to the softmax denominator, and LN is invariant
            to a positive per-row scale)
  out   = Prelu(rstd * (y' - mean(y')))

v3 pipeline (everything built transposed, no PE transposes):
  - P^T = gf_pair_slab.T @ lf streams through the same stationary slab as
    the gf@W1^T matmul (shared LDWEIGHTS, pair = 2 graphs on 128 partitions)
  - norms: DVE squares (4x bf16 mode) + ct-halving adds, then a gpsimd
    partition_all_reduce; rsqrt of the norm outer product via exp(-0.5*ln(x))
    so the scalar engine stays on ONE activation-table set the whole kernel
  - stack^T [128, 100] per graph is written in place: E rows by the Exp
    activation, (s*A_raw) rows by a +-64 partition-shifted multiply
  - LN stats via bn_stats/bn_aggr (DVE); rstd = exp(-0.5*ln(var+eps));
    Prelu(scale=rstd, bias=-mu*rstd) fuses the whole LN tail in one pass
  - output DMA'd as bf16, host casts to f32

When W_adj != I, LN is non-trivial, or b_aff != 0, a general (slower)
fallback kernel is built instead.
"""

import numpy as np
import ml_dtypes

B, T, N, C = 16, 32, 100, 1024
M = 49
MP = 64  # m padded to a 64-row half-slab
BT = B * T
NCORES = 8
GPC = BT // NCORES  # graphs per core (64)
QPC = GPC // 2  # graph pairs per core (32)
CT = C // 128  # contraction tiles (8)
G = 8  # graphs per block
Q = G // 2  # pairs per block
NBLK = GPC // G

_BF16 = ml_dtypes.bfloat16

# per-parity geometry of the legacy fallback stacked matmul
KJ = [2 * M + 1, MP + M]
E_COL = [0, MP]
ARAW_COL = [M, 0]
S_COL = [2 * M, M]


def _build_fast():
    import concourse.bacc as bacc
    import concourse.mybir as mybir
    import concourse.tile as tile
    import concourse.bass_isa as bass_isa

    AF = mybir.ActivationFunctionType
    ALU = mybir.AluOpType
    bf16 = mybir.dt.bfloat16
    f32 = mybir.dt.float32

    nc = bacc.Bacc("TRN2", target_bir_lowering=False, debug=False,
                   num_devices=NCORES)

    # Pre-load the one activation-table set containing every function used
    # (Exp/Ln/Copy/Prelu). The auto-inserter greedily picks the first set per
    # function, which would thrash tables on every Ln<->Exp transition.
    def _preload_act_table():
        from concourse.hw_specs import get_activation_tables
        need = {AF.Exp, AF.Ln, AF.Copy, AF.Prelu}
        tabs = list(get_activation_tables(nc.m.arch).values())
        idx = next(i for i, fs in enumerate(tabs) if need <= fs)
        inst = mybir.InstLoadActFuncSet(
            name=nc.get_next_instruction_name(), ins=[], outs=[],
            act_func_set_id=idx)
        nc.scalar.add_instruction(inst)

    lft = nc.dram_tensor("lft", [GPC, 128, CT, N], bf16, kind="ExternalInput")
    gfp = nc.dram_tensor("gfp", [QPC, 128, CT, 2, MP], bf16,
                         kind="ExternalInput")
    w1t = nc.dram_tensor("w1t", [128, CT, C], bf16, kind="ExternalInput")
    w2tb = nc.dram_tensor("w2tb", [M + 1, C], bf16, kind="ExternalInput")
    out = nc.dram_tensor("out", [GPC, N, C], bf16, kind="ExternalOutput")

    with tile.TileContext(nc) as tc:
        with (
            tc.tile_pool(name="statics", bufs=1) as statics,
            tc.tile_pool(name="blk", bufs=2) as blk,
            tc.tile_pool(name="sq", bufs=1) as sqp,
            tc.tile_pool(name="pairp", bufs=2) as pairp,
            tc.tile_pool(name="gp", bufs=2) as gp,
            tc.tile_pool(name="ps_pw", bufs=1, space="PSUM") as ps_pw,
            tc.tile_pool(name="ps_pt", bufs=1, space="PSUM") as ps_pt,
            tc.tile_pool(name="ps_y", bufs=2, space="PSUM") as ps_y,
            tc.tile_pool(name="ps_s2", bufs=1, space="PSUM") as ps_s2,
        ):
            _preload_act_table()
            onecol = statics.tile([128, 1], bf16)
            nc.gpsimd.memset(onecol[:], 1.0)
            epsln = statics.tile([128, 1], f32)
            nc.gpsimd.memset(epsln[:], 1e-5)
            epstiny = statics.tile([128, 1], f32)
            nc.gpsimd.memset(epstiny[:], 1e-12)
            w1t_sb = statics.tile([128, CT, C], bf16)
            nc.sync.dma_start(w1t_sb[:], w1t.ap())
            # rstk[j][r]: stacked-matmul rhs, 2 parities x 2 rotation buffers.
            #  j=0: rows 0:64 = pw[0:64] (gfW1 even, pads zero), 64:113 = W2T,
            #       113:128 = zero (matching stack rows are zero anyway)
            #  j=1: rows 0:49 = W2T, 49:64 = zero, 64:128 = pw[64:128]
            rstks = []
            for j in range(2):
                row = []
                for r in range(2):
                    t = statics.tile([128, C], bf16, name=f"rstk{j}_{r}",
                                     tag=f"rstk{j}_{r}")
                    if j == 0:
                        nc.gpsimd.memset(t[96:128, :], 0.0)
                        nc.sync.dma_start(t[64:64 + M, :], w2tb.ap()[0:M])
                    else:
                        nc.gpsimd.memset(t[32:64, :], 0.0)
                        nc.sync.dma_start(t[0:M, :], w2tb.ap()[0:M])
                    row.append(t)
                rstks.append(row)

            def emit_loads(bk):
                lftb = blk.tile([128, G, CT, N], bf16, tag="lftb")
                lsrc = lft.ap()[bk * G:(bk + 1) * G]
                nc.sync.dma_start(lftb[:],
                                  lsrc.rearrange("g p t n -> p g t n"))
                gfpb = blk.tile([128, Q, CT, 2, MP], bf16, tag="gfpb")
                gsrc = gfp.ap()[bk * Q:(bk + 1) * Q]
                nc.sync.dma_start(gfpb[:],
                                  gsrc.rearrange("q p t j m -> p q t j m"))
                return lftb, gfpb

            def emit_norms(lftb, gfpb):
                # squares -> ct-halving adds -> replicated partition sums
                sql = sqp.tile([128, G, CT, N], bf16, tag="sql")
                nc.vector.tensor_tensor(out=sql[:], in0=lftb[:], in1=lftb[:],
                                        op=ALU.mult)
                sqg = sqp.tile([128, Q, CT, 2, MP], bf16, tag="sqg")
                nc.vector.tensor_tensor(out=sqg[:], in0=gfpb[:], in1=gfpb[:],
                                        op=ALU.mult)
                sql4 = sqp.tile([128, G, 4, N], bf16, tag="sql4")
                nc.vector.tensor_tensor(out=sql4[:], in0=sql[:, :, 0:4, :],
                                        in1=sql[:, :, 4:8, :], op=ALU.add)
                sql2 = sqp.tile([128, G, 2, N], bf16, tag="sql2")
                nc.vector.tensor_tensor(out=sql2[:], in0=sql4[:, :, 0:2, :],
                                        in1=sql4[:, :, 2:4, :], op=ALU.add)
                sql1 = sqp.tile([128, G, N], bf16, tag="sql1")
                nc.vector.tensor_tensor(out=sql1[:], in0=sql2[:, :, 0, :],
                                        in1=sql2[:, :, 1, :], op=ALU.add)
                sqg4 = sqp.tile([128, Q, 4, 2, MP], bf16, tag="sqg4")
                nc.vector.tensor_tensor(out=sqg4[:], in0=sqg[:, :, 0:4, :, :],
                                        in1=sqg[:, :, 4:8, :, :], op=ALU.add)
                sqg2 = sqp.tile([128, Q, 2, 2, MP], bf16, tag="sqg2")
                nc.vector.tensor_tensor(out=sqg2[:], in0=sqg4[:, :, 0:2, :, :],
                                        in1=sqg4[:, :, 2:4, :, :], op=ALU.add)
                sqg1 = sqp.tile([128, Q, 2, MP], bf16, tag="sqg1")
                nc.vector.tensor_tensor(out=sqg1[:], in0=sqg2[:, :, 0, :, :],
                                        in1=sqg2[:, :, 1, :, :], op=ALU.add)
                sql_ar = blk.tile([128, G, N], bf16, tag="sql_ar")
                nc.gpsimd.partition_all_reduce(
                    sql_ar[:], sql1[:], channels=128,
                    reduce_op=bass_isa.ReduceOp.add)
                sqg_ar = blk.tile([128, Q, 2, MP], bf16, tag="sqg_ar")
                nc.gpsimd.partition_all_reduce(
                    sqg_ar[:], sqg1[:], channels=128,
                    reduce_op=bass_isa.ReduceOp.add)
                return sql_ar, sqg_ar

            cur = emit_loads(0)
            cur_norms = emit_norms(*cur)
            for bk in range(NBLK):
                lftb, gfpb = cur
                sql_ar, sqg_ar = cur_norms

                yo_blk = blk.tile([N, G, C], bf16, tag="yo_blk")

                for q in range(Q):
                    # ---- gfW1 + P^T through one stationary slab ----
                    # gfW1 is accumulated in two 512-col halves from a
                    # double-buffered 1-bank pool, so the next accumulation
                    # group can start while the previous half is copied out.
                    # ptq and outer share one PSUM bank (512 f32 cols)
                    ptb = ps_pt.tile([128, 512], f32, tag="ptb")
                    ptq = ptb[:, 0:200].rearrange("p (a b) -> p a b", a=2)
                    outer = ptb[:, 256:456].rearrange("p (a b) -> p a b", a=2)
                    pw = ps_pw.tile([128, C], f32, tag="pw")
                    for ct in range(CT):
                        slab = gfpb[:, q, ct, :, :]
                        st = (ct == 0)
                        sp = (ct == CT - 1)
                        nc.tensor.matmul(pw[:, 0:512], slab,
                                         w1t_sb[:, ct, 0:512],
                                         start=st, stop=sp)
                        nc.tensor.matmul(pw[:, 512:1024], slab,
                                         w1t_sb[:, ct, 512:1024],
                                         start=st, stop=sp)
                        nc.tensor.matmul(ptq, slab,
                                         lftb[:, 2 * q:2 * q + 2, ct, :],
                                         start=st, stop=sp)
                    # gfW1 rows into the stacked rhs (PSUM -> SBUF bf16)
                    with tc.high_priority():
                        nc.scalar.activation(rstks[0][q % 2][0:64, :],
                                             pw[0:64, :], AF.Copy)
                        nc.scalar.activation(rstks[1][q % 2][64:128, :],
                                             pw[64:128, :], AF.Copy)

                    # ---- rs = rsqrt(outer(s_g, s_l)) ----
                    nc.tensor.matmul(outer, sqg_ar[0:1, q, :, :],
                                     sql_ar[0:1, 2 * q:2 * q + 2, :],
                                     start=True, stop=True)
                    lnsc = pairp.tile([128, 2, N], f32, tag="lnsc")
                    nc.scalar.activation(lnsc[:], outer, AF.Ln,
                                         bias=epstiny[:])
                    rs_full = pairp.tile([128, 2, N], bf16, tag="rs")
                    nc.scalar.activation(rs_full[:], lnsc[:], AF.Exp,
                                         scale=-0.5)
                    araw = pairp.tile([128, 2, N], bf16, tag="araw")
                    nc.vector.tensor_tensor(out=araw[:], in0=ptq,
                                            in1=rs_full[:], op=ALU.mult)

                    # ---- stacks for both graphs ----
                    S = pairp.tile([128, 2, N], bf16, tag="stack")
                    s2 = ps_s2.tile([1, 2, N], f32, tag="s2")
                    srow = gp.tile([1, 2, N], bf16, tag="srow")
                    sbc = gp.tile([128, 2, N], bf16, tag="sbc")
                    stats2 = gp.tile([N, 2, 2, 6], f32, tag="stats2")
                    mv2 = gp.tile([N, 2, 2], f32, tag="mv2")
                    lnv = gp.tile([N, 2], f32, tag="lnv")
                    rstd2 = gp.tile([N, 2], f32, tag="rstd2")
                    negmu2 = gp.tile([N, 2], f32, tag="negmu2")

                    for j in range(2):
                        jb = 64 * j
                        ob = 64 - jb
                        nc.scalar.activation(S[jb:jb + 64, j, :],
                                             araw[jb:jb + 64, j, :],
                                             AF.Exp, scale=5.0)
                        # sum only the 49 real bins: pad rows hold exp(0)=1
                        nc.tensor.matmul(s2[0:1, j, :], onecol[jb:jb + M, :],
                                         S[jb:jb + M, j, :],
                                         start=True, stop=True)
                        nc.vector.tensor_copy(srow[0:1, j, :],
                                              s2[0:1, j, :])
                        nc.gpsimd.partition_broadcast(sbc[:, j, :],
                                                      srow[0:1, j, :],
                                                      channels=128)
                        nc.gpsimd.tensor_tensor(out=S[ob:ob + 64, j, :],
                                                in0=araw[jb:jb + 64, j, :],
                                                in1=sbc[jb:jb + 64, j, :],
                                                op=ALU.mult)

                    # ---- y = stack^T.T @ rstk, then fused LN tail ----
                    for j in range(2):
                        gi = 2 * q + j
                        yps = ps_y.tile([N, C], f32, tag="y")
                        rstk_t = rstks[j][q % 2]
                        for h in range(2):
                            nc.tensor.matmul(yps[:, h * 512:(h + 1) * 512],
                                             S[:, j, :],
                                             rstk_t[:, h * 512:(h + 1) * 512],
                                             start=True, stop=True)
                        yv = yps[:].rearrange("p (a b) -> p a b", a=2)
                        nc.vector.bn_stats(out=stats2[:, j, 0, :],
                                           in_=yv[:, 0, :])
                        nc.vector.bn_stats(out=stats2[:, j, 1, :],
                                           in_=yv[:, 1, :])
                        nc.vector.bn_aggr(out=mv2[:, j, :],
                                          in_=stats2[:, j, :, :])
                        nc.scalar.activation(lnv[:, j:j + 1],
                                             mv2[:, j, 1:2], AF.Ln,
                                             bias=epsln[0:N])
                        nc.scalar.activation(rstd2[:, j:j + 1],
                                             lnv[:, j:j + 1], AF.Exp,
                                             scale=-0.5)
                        nc.vector.tensor_scalar(
                            out=negmu2[:, j:j + 1], in0=mv2[:, j, 0:1],
                            scalar1=rstd2[:, j:j + 1], scalar2=-1.0,
                            op0=ALU.mult, op1=ALU.mult)
                        nc.scalar.activation(yo_blk[:, gi, :], yps[:],
                                             AF.Prelu,
                                             bias=negmu2[:, j:j + 1],
                                             scale=rstd2[:, j:j + 1],
                                             alpha=0.01)

                    # software pipeline: emit the next block's loads and
                    # norm chain mid-block so they fill engine gaps
                    if q == 1 and bk + 1 < NBLK:
                        cur = emit_loads(bk + 1)
                        cur_norms = emit_norms(*cur)

                # two half-block output DMAs so the first can overlap the
                # second half's compute
                for hb in range(2):
                    g0 = hb * (G // 2)
                    dsts = out.ap()[bk * G + g0:bk * G + g0 + G // 2]
                    nc.sync.dma_start(dsts.rearrange("g n c -> n g c"),
                                      yo_blk[:, g0:g0 + G // 2, :])

    nc.compile()
    return nc


def _build_fallback(general_w: bool, general_ln: bool):
    import concourse.bacc as bacc
    import concourse.mybir as mybir
    import concourse.tile as tile
    from concourse import masks

    AF = mybir.ActivationFunctionType
    ALU = mybir.AluOpType
    bf16 = mybir.dt.bfloat16
    f32 = mybir.dt.float32

    nc = bacc.Bacc("TRN2", target_bir_lowering=False, debug=False,
                   num_devices=NCORES)

    lft = nc.dram_tensor("lft", [GPC, 128, CT, N], bf16, kind="ExternalInput")
    gfp = nc.dram_tensor("gfp", [QPC, 128, CT, 2, MP], bf16,
                         kind="ExternalInput")
    w1t = nc.dram_tensor("w1t", [128, CT, C], bf16, kind="ExternalInput")
    w2tb = nc.dram_tensor("w2tb", [M + 1, C], bf16, kind="ExternalInput")
    if general_w:
        wadjt = nc.dram_tensor("wadjt", [128, CT, CT, 128], bf16,
                               kind="ExternalInput")
    if general_ln:
        grep = nc.dram_tensor("grep", [128, C], f32, kind="ExternalInput")
        brep = nc.dram_tensor("brep", [128, C], f32, kind="ExternalInput")
    out = nc.dram_tensor("out", [GPC, N, C], f32, kind="ExternalOutput")

    with tile.TileContext(nc) as tc:
        with (
            tc.tile_pool(name="statics", bufs=1) as statics,
            tc.tile_pool(name="pair_sb", bufs=2) as pair_sb,
            tc.tile_pool(name="graph_sb", bufs=3) as graph_sb,
            tc.tile_pool(name="ps_small", bufs=2, space="PSUM") as ps_small,
            tc.tile_pool(name="ps_pair", bufs=1, space="PSUM") as ps_pair,
            tc.tile_pool(name="ps_y", bufs=1 if general_w else 2,
                         space="PSUM") as ps_y,
        ):
            ident = statics.tile([128, 128], f32)
            masks.make_identity(nc, ident[:])
            onecol = statics.tile([128, 1], bf16)
            nc.gpsimd.memset(onecol[:], 1.0)
            epsln = statics.tile([128, 1], f32)
            nc.gpsimd.memset(epsln[:], 1e-5)
            w1t_sb = statics.tile([128, CT, C], bf16)
            nc.sync.dma_start(w1t_sb[:], w1t.ap())
            rstk0 = statics.tile([2 * M + 1, C], bf16)
            nc.sync.dma_start(rstk0[M:2 * M + 1, :], w2tb.ap())
            rstk1 = statics.tile([MP + M, C], bf16)
            nc.gpsimd.memset(rstk1[0:MP, :], 0.0)
            nc.sync.dma_start(rstk1[0:M + 1, :], w2tb.ap())
            rstk = [rstk0, rstk1]
            if general_w:
                wadj_sb = statics.tile([128, CT, CT, 128], bf16)
                nc.sync.dma_start(wadj_sb[:], wadjt.ap())
            if general_ln:
                grep_sb = statics.tile([128, C], f32)
                brep_sb = statics.tile([128, C], f32)
                nc.sync.dma_start(grep_sb[:], grep.ap())
                nc.sync.dma_start(brep_sb[:], brep.ap())

            for q in range(QPC):
                gfp_t = pair_sb.tile([128, CT, 2, MP], bf16, tag="gfp")
                nc.sync.dma_start(gfp_t[:], gfp.ap()[q])

                pw = ps_pair.tile([128, C], f32, tag="pw")
                for ct in range(CT):
                    for h in range(2):
                        nc.tensor.matmul(
                            pw[:, h * 512:(h + 1) * 512],
                            gfp_t[:, ct, :, :],
                            w1t_sb[:, ct, h * 512:(h + 1) * 512],
                            start=(ct == 0), stop=(ct == CT - 1))

                if general_w:
                    qps = ps_pair.tile([128, CT, 2, MP], f32, tag="qps")
                    for dt_i in range(CT):
                        for ct in range(CT):
                            nc.tensor.matmul(
                                qps[:, dt_i, :, :],
                                wadj_sb[:, ct, dt_i, :],
                                gfp_t[:, ct, :, :],
                                start=(ct == 0), stop=(ct == CT - 1))
                    qp_sb = pair_sb.tile([128, CT, 2, MP], bf16, tag="qp")
                    nc.scalar.activation(qp_sb[:], qps[:], AF.Copy)
                    rhs_pm = qp_sb
                else:
                    rhs_pm = gfp_t

                sqg = pair_sb.tile([128, CT, 2, MP], bf16, tag="sqg")
                nc.vector.tensor_tensor(
                    out=sqg[:], in0=gfp_t[:], in1=gfp_t[:], op=ALU.mult)
                rg_ps = ps_small.tile([128, 512], f32, tag="sm")
                for ct in range(CT):
                    nc.tensor.matmul(
                        rg_ps[0:1, 0:2 * MP], onecol[:], sqg[:, ct, :, :],
                        start=(ct == 0), stop=(ct == CT - 1))
                rg_f = pair_sb.tile([1, 2, MP], f32, tag="rgf")
                nc.vector.reciprocal(rg_f[:, 0, 0:M], rg_ps[0:1, 0:M])
                nc.vector.reciprocal(rg_f[:, 1, 0:M],
                                     rg_ps[0:1, MP:MP + M])
                rg_row = pair_sb.tile([1, 2, MP], bf16, tag="rgr")
                nc.scalar.activation(rg_row[:, 0, 0:M], rg_f[:, 0, 0:M],
                                     AF.Sqrt)
                nc.scalar.activation(rg_row[:, 1, 0:M], rg_f[:, 1, 0:M],
                                     AF.Sqrt)

                for j in range(2):
                    g = 2 * q + j
                    kj = KJ[j]
                    lft_t = graph_sb.tile([128, CT, N], bf16, tag="lft")
                    nc.sync.dma_start(lft_t[:], lft.ap()[g])

                    sql = graph_sb.tile([128, CT, N], bf16, tag="sql")
                    nc.vector.tensor_tensor(
                        out=sql[:], in0=lft_t[:], in1=lft_t[:], op=ALU.mult)
                    sm = ps_small.tile([128, 512], f32, tag="sm")
                    for ct in range(CT):
                        nc.tensor.matmul(
                            sm[0:1, 256:256 + N], onecol[:], sql[:, ct, :],
                            start=(ct == 0), stop=(ct == CT - 1))
                    sl_f = graph_sb.tile([1, N], f32, tag="slf")
                    nc.vector.reciprocal(sl_f[:], sm[0:1, 256:256 + N])
                    rl_row = graph_sb.tile([1, N], bf16, tag="rlr")
                    nc.scalar.activation(rl_row[:], sl_f[:], AF.Sqrt)

                    nc.tensor.matmul(
                        sm[0:N, 64:64 + M], rl_row[:],
                        rg_row[:, j, 0:M], start=True, stop=True)
                    s_sb = graph_sb.tile([N, M], f32, tag="s_sb")
                    nc.scalar.activation(s_sb[:], sm[0:N, 64:64 + M],
                                         AF.Copy)

                    for ct in range(CT):
                        nc.tensor.matmul(
                            sm[0:N, 0:M], lft_t[:, ct, :],
                            rhs_pm[:, ct, j, 0:M],
                            start=(ct == 0), stop=(ct == CT - 1))

                    stack = graph_sb.tile([N, 128], f32, tag="stack")
                    araw = stack[:, ARAW_COL[j]:ARAW_COL[j] + M]
                    nc.vector.tensor_tensor(
                        out=araw, in0=sm[0:N, 0:M], in1=s_sb[:],
                        op=ALU.mult)
                    nc.gpsimd.memset(stack[:, S_COL[j]:S_COL[j] + 1], 1.0)
                    if j == 1:
                        nc.gpsimd.memset(stack[:, M + 1:MP], 0.0)

                    e_t = graph_sb.tile([N, M], f32, tag="e")
                    ssum = graph_sb.tile([N, 1], f32, tag="ssum")
                    nc.scalar.activation(
                        e_t[:], araw, AF.Exp, scale=5.0, accum_out=ssum[:])
                    sinv = graph_sb.tile([N, 1], f32, tag="sinv")
                    nc.vector.reciprocal(sinv[:], ssum[:])
                    nc.vector.tensor_scalar(
                        out=stack[:, E_COL[j]:E_COL[j] + M], in0=e_t[:],
                        scalar1=sinv[:], scalar2=None, op0=ALU.mult)

                    ident_b = graph_sb.tile([128, 128], bf16, tag="idb")
                    nc.vector.tensor_copy(ident_b[0:N, 0:N], ident[0:N, 0:N])
                    stack_b = graph_sb.tile([N, 128], bf16, tag="stackb")
                    nc.vector.tensor_copy(stack_b[:, 0:kj], stack[:, 0:kj])
                    nc.tensor.transpose(
                        sm[0:kj, 128:128 + N], stack_b[:, 0:kj],
                        ident_b[0:N, 0:N])
                    lhs_y = graph_sb.tile([128, N], bf16, tag="lhy")
                    nc.scalar.activation(
                        lhs_y[0:kj, :], sm[0:kj, 128:128 + N], AF.Copy)

                    if j == 0:
                        nc.scalar.activation(
                            rstk0[0:M, :], pw[0:M, :], AF.Copy)
                    else:
                        nc.scalar.activation(
                            rstk1[MP:MP + M, :], pw[MP:MP + M, :], AF.Copy)

                    yps = ps_y.tile([N, C], f32, tag="y")
                    for h in range(2):
                        nc.tensor.matmul(
                            yps[:, h * 512:(h + 1) * 512], lhs_y[0:kj, :],
                            rstk[j][:, h * 512:(h + 1) * 512],
                            start=True, stop=True)

                    stats = graph_sb.tile([N, 2, 6], f32, tag="stats")
                    yps_v = yps[:].rearrange("p (a b) -> p a b", a=2)
                    nc.vector.bn_stats(out=stats[:, 0, :], in_=yps_v[:, 0, :])
                    nc.vector.bn_stats(out=stats[:, 1, :], in_=yps_v[:, 1, :])
                    mv = graph_sb.tile([N, 2], f32, tag="mv")
                    nc.vector.bn_aggr(out=mv[:], in_=stats[:])
                    rstd = graph_sb.tile([N, 1], f32, tag="rstd")
                    nc.scalar.activation(
                        rstd[:], mv[:, 1:2], AF.Sqrt, bias=epsln[0:N])
                    nc.vector.reciprocal(rstd[:], rstd[:])
                    negmurs = graph_sb.tile([N, 1], f32, tag="negmurs")
                    nc.vector.tensor_scalar(
                        out=negmurs[:], in0=mv[:, 0:1], scalar1=rstd[:],
                        scalar2=-1.0, op0=ALU.mult, op1=ALU.mult)

                    y_out = graph_sb.tile([N, C], f32, tag="yo")
                    if general_ln:
                        nc.scalar.activation(
                            y_out[:], yps[:], AF.Copy, bias=negmurs[:],
                            scale=rstd[:])
                        nc.vector.tensor_tensor(
                            out=y_out[:], in0=y_out[:], in1=grep_sb[0:N, :],
                            op=ALU.mult)
                        nc.vector.tensor_tensor(
                            out=y_out[:], in0=y_out[:], in1=brep_sb[0:N, :],
                            op=ALU.add)
                        nc.scalar.activation(
                            y_out[:], y_out[:], AF.Lrelu, alpha=0.01)
                    else:
                        nc.scalar.activation(
                            y_out[:], yps[:], AF.Lrelu, bias=negmurs[:],
                            scale=rstd[:], alpha=0.01)
                    nc.sync.dma_start(out.ap()[g], y_out[:])

    nc.compile()
    return nc


_cache = {}


def _get_nc(general_w: bool, general_ln: bool, general_b: bool = False):
    key = (general_w, general_ln, general_b)
    if key not in _cache:
        if general_w or general_ln or general_b:
            _cache[key] = _build_fallback(general_w, general_ln)
        else:
            _cache[key] = _build_fast()
    return _cache[key]


def _pack_inputs(local_feat, global_feat, W_aff, b_aff):
    lf = np.ascontiguousarray(local_feat.reshape(BT, N, C))
    gf = np.ascontiguousarray(global_feat.reshape(BT, M, C))
    # lft[g, p, t, n] = lf[g, n, t*128+p]
    lft = lf.transpose(0, 2, 1).reshape(BT, CT, 128, N).transpose(0, 2, 1, 3)
    lft = np.ascontiguousarray(lft.astype(_BF16))
    # gfp[q, p, t, j, m] = gf[2q+j, m, t*128+p], m zero-padded 49 -> 64
    gfp = np.zeros((BT // 2, 128, CT, 2, MP), dtype=_BF16)
    g4 = gf.transpose(0, 2, 1).reshape(BT // 2, 2, CT, 128, M)
    gfp[:, :, :, :, 0:M] = g4.transpose(0, 3, 2, 1, 4).astype(_BF16)
    # w1t[p, t, co] = W_aff[co, t*128+p]
    w1t = np.ascontiguousarray(
        W_aff[:, :C].T.reshape(CT, 128, C).transpose(1, 0, 2).astype(_BF16))
    # w2tb rows 0:49 = W2^T, row 49 = b_aff
    w2tb = np.concatenate([W_aff[:, C:C + M].T, b_aff[None, :]], axis=0)
    w2tb = np.ascontiguousarray(w2tb.astype(_BF16))
    return lft, gfp, w1t, w2tb


def _make_in_maps(lft, gfp, w1t, w2tb, extra):
    shared = {"w1t": w1t, "w2tb": w2tb, **extra}
    in_maps = []
    for k in range(NCORES):
        gs = slice(k * GPC, (k + 1) * GPC)
        qs = slice(k * QPC, (k + 1) * QPC)
        in_maps.append({"lft": np.ascontiguousarray(lft[gs]),
                        "gfp": np.ascontiguousarray(gfp[qs]), **shared})
    return in_maps


def kernel(local_feat, global_feat, pos, W_adj, W_aff, b_aff, ln_g, ln_b):
    from concourse.bass_utils import run_bass_kernel_spmd

    general_w = not np.array_equal(W_adj, np.eye(C, dtype=W_adj.dtype))
    general_ln = not (np.all(ln_g == 1.0) and np.all(ln_b == 0.0))
    general_b = bool(np.any(b_aff != 0.0))

    lft, gfp, w1t, w2tb = _pack_inputs(local_feat, global_feat, W_aff, b_aff)

    extra = {}
    if general_w:
        # wadjt[p, ct, dt, d] = W_adj[dt*128+d, ct*128+p]
        wadjt = W_adj.T.reshape(CT, 128, CT, 128).transpose(1, 0, 2, 3)
        extra["wadjt"] = np.ascontiguousarray(wadjt.astype(_BF16))
    if general_ln:
        extra["grep"] = np.ascontiguousarray(
            np.broadcast_to(ln_g[None, :], (128, C)).astype(np.float32))
        extra["brep"] = np.ascontiguousarray(
            np.broadcast_to(ln_b[None, :], (128, C)).astype(np.float32))

    nc = _get_nc(general_w, general_ln, general_b)
    in_maps = _make_in_maps(lft, gfp, w1t, w2tb, extra)

    res = run_bass_kernel_spmd(nc, in_maps, core_ids=list(range(NCORES)))
    y = np.concatenate([res.results[k]["out"] for k in range(NCORES)], axis=0)
    return np.ascontiguousarray(y.reshape(B, T, N, C).astype(np.float32))

